# revision 1
# baseline (speedup 1.0000x reference)
"""Causal linear attention (elu+1 feature map) for Trainium2, 8 NeuronCores.

Problem: B=2, S=2048, D=1024, H=16, HD=64.
  q/k/v projections [S,D]@[D,H*HD], phi = elu+1, causal linear attention
  out[t] = (sum_{i<=t} (phi_q[t].phi_k[i]) v[i]) / (phi_q[t].sum_{i<=t} phi_k[i] + eps)

Sharding: core c -> (batch b=c//4, heads h0=4*(c%4) .. h0+3). No cross-core comm.
Host feeds x^T [D,S] per core (layout choice for the NEFF inputs) so the
contraction dim d sits on SBUF partitions with no on-chip transposes.

Device algorithm (per core, 4 heads, all fp32):
  - proj q,k -> phi_qT/phi_kT [64,2048] per head (head-pairs packed on 128 parts)
  - phi_k seq layout via PE transpose of phi_kT
  - v projected seq-major with an appended ones column (v_aug [128,65] per chunk)
  - chunked attention, L=128: A_T = phi_kT_c^T-free matmul -> mask (j<=t) ->
    out_psum = tril(A)^T-form matmul @ v_aug + phi_q_c @ S_prev ; S += phi_k_c^T @ v_aug
    The ones column of v_aug makes column 64 of out_psum the normalizer.
"""

import os
import threading

import numpy as np

B, S, D, H, HD = 2, 2048, 1024, 16, 64
EPS = 1e-6
N_CORES = 8
HPC = 4            # heads per core
HDC = HPC * HD     # 256 projected cols per core
NCHUNK = S // 128  # 16
F32 = None         # set after import

_lock = threading.Lock()
_cache = {}


def _build_nc(dump=False):
    import concourse.bass as bass
    import concourse.tile as tile
    from concourse import bacc, mybir

    f32 = mybir.dt.float32
    Alu = mybir.AluOpType
    Act = mybir.ActivationFunctionType

    nc = bacc.Bacc("TRN2", target_bir_lowering=False, debug=False)
    f32r = mybir.dt.float32r

    def R(ap):
        # fp32 data reinterpreted as float32r: full-rate PE streaming
        return ap.bitcast(f32r)

    xqT = nc.dram_tensor("xqT", [D, S], f32r, kind="ExternalInput").ap()
    xkT = nc.dram_tensor("xkT", [D, S], f32r, kind="ExternalInput").ap()
    xvT = nc.dram_tensor("xvT", [D, S], f32r, kind="ExternalInput").ap()
    wq = nc.dram_tensor("wq", [D, HDC], f32r, kind="ExternalInput").ap()
    wk = nc.dram_tensor("wk", [D, HDC], f32r, kind="ExternalInput").ap()
    wv = nc.dram_tensor("wv", [D, HDC], f32r, kind="ExternalInput").ap()
    out = nc.dram_tensor("out", [S, HDC], f32, kind="ExternalOutput").ap()
    if dump:
        d_phi_qT = [nc.dram_tensor(f"d_phi_qT{i}", [128, S], f32, kind="ExternalOutput").ap() for i in range(2)]
        d_phi_kT = [nc.dram_tensor(f"d_phi_kT{i}", [128, S], f32, kind="ExternalOutput").ap() for i in range(2)]
        d_phi_ks = nc.dram_tensor("d_phi_ks", [128, NCHUNK * HDC], f32, kind="ExternalOutput").ap()
        d_v_aug = nc.dram_tensor("d_v_aug", [128, NCHUNK * HPC * 65], f32, kind="ExternalOutput").ap()

    DC = D // 128  # 8 contraction chunks

    with tile.TileContext(nc) as tc:
        with (
            tc.tile_pool(name="consts", bufs=1) as consts,
            tc.tile_pool(name="weights", bufs=1) as wpool,
            tc.tile_pool(name="resident", bufs=1) as res,
            tc.tile_pool(name="xin", bufs=30) as xin,
            tc.tile_pool(name="work", bufs=3) as work,
            tc.tile_pool(name="attn", bufs=3) as attn,
            tc.tile_pool(name="psum", bufs=2, space="PSUM") as psum,
        ):
            # ---- constants ----
            ones = consts.tile([128, 128], f32)
            nc.vector.memset(ones[:], 1.0)
            # identity (two 64x64 diagonal blocks are slices of the 128x128 I)
            ident = consts.tile([128, 128], f32)
            nc.gpsimd.affine_select(
                ident[:], ones[:], pattern=[[-1, 128]], base=0,
                channel_multiplier=1, compare_op=Alu.is_equal, fill=0.0,
            )
            # causal mask in [j (part), t (free)] layout: keep j <= t
            maskT = consts.tile([128, 128], f32)
            nc.gpsimd.affine_select(
                maskT[:], ones[:], pattern=[[1, 128]], base=0,
                channel_multiplier=-1, compare_op=Alu.is_ge, fill=0.0,
            )

            # ---- weights: [D, HDC] -> [128, DC, HDC] (partition = d % 128) ----
            w_sb = {}
            for name, wdram in (("q", wq), ("k", wk), ("v", wv)):
                wt = wpool.tile([128, DC, HDC], f32r, name=f"w{name}_sb")
                nc.sync.dma_start(wt[:], wdram.rearrange("(dc p) m -> p dc m", p=128))
                w_sb[name] = wt

            # ---- resident activations ----
            # head pairs hp=0 (heads 0,1) / hp=1 (heads 2,3), head at partition 64*(h%2)
            phi_qT = [res.tile([128, S], f32, name=f"phi_qT{i}") for i in range(2)]
            phi_kT = [res.tile([128, S], f32, name=f"phi_kT{i}") for i in range(2)]
            # seq-major: [s-in-chunk, (chunk, head, :)]
            phi_ks = res.tile([128, NCHUNK * HDC], f32, name="phi_ks")
            v_aug = res.tile([128, NCHUNK * HPC * 65], f32, name="v_aug")
            # ones column of v_aug (written once; v copies fill the rest)
            nc.vector.memset(v_aug.rearrange("p (c h e) -> p c h e", c=NCHUNK, h=HPC)[:, :, :, 64:65], 1.0)

            # ---- load x^T tiles (streamed by s-half to bound SBUF) ----
            def load_half(xdram, qt, tag):
                tiles = []
                for dc in range(DC):
                    t = xin.tile([128, S // 4], f32r, name=f"x_{tag}_{qt}_{dc}", tag="xin")
                    nc.sync.dma_start(t[:], xdram[dc * 128:(dc + 1) * 128, qt * (S // 4):(qt + 1) * (S // 4)])
                    tiles.append(t)
                return tiles

            def phi_from_psum(ps, dst, n):
                # phi(x) = exp(min(x,0)) + max(x,0); m' = relu(-x); e = exp(-m')
                t1 = work.tile([128, n], f32, tag="phi1")
                nc.scalar.activation(t1[:], ps[:], Act.Relu, scale=-1.0)
                t2 = work.tile([128, n], f32, tag="phi2")
                nc.scalar.activation(t2[:], t1[:], Act.Exp, scale=-1.0)
                nc.vector.scalar_tensor_tensor(
                    dst, ps[:], 0.0, t2[:], op0=Alu.max, op1=Alu.add)

            # ---- per-half pipeline: proj q/k/v -> phi_k transposes -> attention ----
            S_prev = []
            for hp in range(2):
                s0t = res.tile([128, 65], f32, name=f"S_init{hp}")
                nc.vector.memset(s0t[:], 0.0)
                S_prev.append(s0t)
            vaug4 = v_aug.rearrange("p (c h e) -> p c h e", c=NCHUNK, h=HPC)

            for qt in range(4):
                # q/k projections for this quarter (one 512-wide col group)
                for tname, dst in (("q", phi_qT), ("k", phi_kT)):
                    xt = load_half({"q": xqT, "k": xkT}[tname], qt, tname)
                    for hp in range(2):
                        s0 = qt * 512
                        ps = psum.tile([128, 512], f32, tag="proj", name=f"ps_{tname}_{qt}_{hp}")
                        for dc in range(DC):
                            nc.tensor.matmul(
                                ps[:], w_sb[tname][:, dc, hp * 128:(hp + 1) * 128],
                                xt[dc][:],
                                start=(dc == 0), stop=(dc == DC - 1),
                            )
                        phi_from_psum(ps, dst[hp][:, s0:s0 + 512], 512)

                # v projection for this quarter (seq-major)
                xt = load_half(xvT, qt, "v")
                for cc in range(NCHUNK // 4):
                    c = qt * (NCHUNK // 4) + cc
                    ps = psum.tile([128, HDC], f32, tag="proj", name=f"ps_v_{c}")
                    for dc in range(DC):
                        nc.tensor.matmul(
                            ps[:], xt[dc][:, cc * 128:(cc + 1) * 128],
                            w_sb["v"][:, dc, :],
                            start=(dc == 0), stop=(dc == DC - 1),
                        )
                    dstv = vaug4[:, c, :, 0:64]
                    nc.any.tensor_copy(dstv, ps.rearrange("p (h e) -> p h e", h=HPC)[:])

                # phi_k seq-major via PE transpose (this quarter's chunks)
                for cc in range(NCHUNK // 4):
                    c = qt * (NCHUNK // 4) + cc
                    for h in range(HPC):
                        hp, hb = h // 2, 64 * (h % 2)
                        tp = psum.tile([128, 64], f32, tag="proj", bufs=2, name=f"tp_{c}_{h}")
                        nc.tensor.transpose(
                            tp[:], phi_kT[hp][hb:hb + 64, c * 128:(c + 1) * 128],
                            ident[hb:hb + 64, hb:hb + 64],
                        )
                        nc.any.tensor_copy(phi_ks[:, c * HDC + h * 64: c * HDC + (h + 1) * 64], tp[:])

                # attention for this quarter's chunks
                for cc in range(NCHUNK // 4):
                    c = qt * (NCHUNK // 4) + cc
                    o_ps = {}
                    for h in range(HPC):
                        hp, hb = h // 2, 64 * (h % 2)
                        kT_c = phi_kT[hp][hb:hb + 64, c * 128:(c + 1) * 128]
                        qT_c = phi_qT[hp][hb:hb + 64, c * 128:(c + 1) * 128]
                        a_ps = psum.tile([128, 128], f32, tag="A", name=f"a_ps_{c}_{h}")
                        nc.tensor.matmul(a_ps[:], kT_c, qT_c, start=True, stop=True)
                        a_sb = attn.tile([128, 128], f32, tag="Asb", name=f"a_sb_{c}_{h}", bufs=6)
                        nc.vector.tensor_tensor(a_sb[:], a_ps[:], maskT[:], op=Alu.mult)
                        op = psum.tile([128, 65], f32, tag="o", name=f"o_ps_{c}_{h}")
                        nc.tensor.matmul(op[:], a_sb[:], vaug4[:, c, h, :],
                                         start=True, stop=(c == 0))
                        if c > 0:
                            nc.tensor.matmul(op[:], qT_c, S_prev[hp][hb:hb + 64, :],
                                             start=False, stop=True)
                        o_ps[h] = op

                    S_new = []
                    for hp in range(2):
                        s_inc = psum.tile([128, 130], f32, tag="Sinc", name=f"s_inc_{c}_{hp}")
                        nc.tensor.matmul(
                            s_inc[:],
                            phi_ks[:, c * HDC + hp * 128: c * HDC + (hp + 1) * 128],
                            vaug4[:, c, 2 * hp:2 * hp + 2, :],
                            start=True, stop=True,
                        )
                        sn = attn.tile([128, 65], f32, tag=f"S{hp}", name=f"S_{c}_{hp}", bufs=2)
                        nc.vector.tensor_tensor(sn[0:64, :], S_prev[hp][0:64, :], s_inc[0:64, 0:65], op=Alu.add)
                        nc.vector.tensor_tensor(sn[64:128, :], S_prev[hp][64:128, :], s_inc[64:128, 65:130], op=Alu.add)
                        S_new.append(sn)
                    S_prev = S_new

                    o_sb = attn.tile([128, HDC], f32, tag="osb", name=f"o_sb_{c}")
                    for h in range(HPC):
                        op = o_ps[h]
                        den = attn.tile([128, 1], f32, tag="den", name=f"den_{c}_{h}", bufs=4)
                        nc.vector.tensor_scalar(den[:], op[:, 64:65], EPS, None, op0=Alu.add)
                        rcp = attn.tile([128, 1], f32, tag="rcp", name=f"rcp_{c}_{h}", bufs=4)
                        nc.vector.reciprocal(rcp[:], den[:])
                        nc.vector.tensor_scalar(o_sb[:, h * 64:(h + 1) * 64], op[:, 0:64],
                                                rcp[:], None, op0=Alu.mult)
                    nc.sync.dma_start(out[c * 128:(c + 1) * 128, :], o_sb[:])

            if dump:
                for i in range(2):
                    nc.sync.dma_start(d_phi_qT[i][:], phi_qT[i][:])
                    nc.sync.dma_start(d_phi_kT[i][:], phi_kT[i][:])
                nc.sync.dma_start(d_phi_ks[:], phi_ks[:])
                nc.sync.dma_start(d_v_aug[:], v_aug[:])

    nc.compile()
    return nc


def _get_nc():
    with _lock:
        if "nc" not in _cache:
            _cache["nc"] = _build_nc()
        return _cache["nc"]


def kernel(query, key, value, query_kernel, key_kernel, value_kernel):
    from concourse.bass_utils import run_bass_kernel_spmd

    nc = _get_nc()

    xT = {}
    for b in range(B):
        xT[("q", b)] = np.ascontiguousarray(query[b].T, dtype=np.float32)
        xT[("k", b)] = np.ascontiguousarray(key[b].T, dtype=np.float32)
        xT[("v", b)] = np.ascontiguousarray(value[b].T, dtype=np.float32)

    in_maps = []
    for c in range(N_CORES):
        b, h0 = c // 4, 4 * (c % 4)
        in_maps.append({
            "xqT": xT[("q", b)],
            "xkT": xT[("k", b)],
            "xvT": xT[("v", b)],
            "wq": np.ascontiguousarray(query_kernel[:, h0:h0 + HPC, :].reshape(D, HDC), dtype=np.float32),
            "wk": np.ascontiguousarray(key_kernel[:, h0:h0 + HPC, :].reshape(D, HDC), dtype=np.float32),
            "wv": np.ascontiguousarray(value_kernel[:, h0:h0 + HPC, :].reshape(D, HDC), dtype=np.float32),
        })

    results = run_bass_kernel_spmd(nc, in_maps, core_ids=list(range(N_CORES)))

    # The reference ends with a FLAT reshape of [B*H, S, HD] -> (B, S, H*HD):
    # output rows [128h:128h+128] of batch b are head h's [S, HD] attention
    # output flat-reshaped to [128, H*HD].
    full = np.empty((B, S, H * HD), dtype=np.float32)
    for c in range(N_CORES):
        b, h0 = c // 4, 4 * (c % 4)
        av = results.results[c]["out"].reshape(S, HPC, HD)
        for hl in range(HPC):
            full[b, (h0 + hl) * 128:(h0 + hl + 1) * 128, :] = (
                av[:, hl, :].reshape(128, H * HD))
    return full



# revision 11
# speedup vs baseline: 1.4500x; 1.4500x over previous
"""Causal linear attention (elu+1 feature map) for Trainium2, 8 NeuronCores.

Problem: B=2, S=2048, D=1024, H=16, HD=64.
  q/k/v projections [S,D]@[D,H*HD], phi = elu+1, causal linear attention
  out[t] = (sum_{i<=t} (phi_q[t].phi_k[i]) v[i]) / (phi_q[t].sum_{i<=t} phi_k[i] + eps)

Sharding: core c -> (batch b=c//4, heads h0=4*(c%4) .. h0+3). No cross-core comm.
Host feeds x^T [D,S] per core in bf16 so the contraction dim d sits on SBUF
partitions with no on-chip transposes, and DMA bytes are halved.

Device algorithm (per core, 4 heads, bf16 matmuls / fp32 psum):
  - proj q,k -> phi_qT/phi_kT [64,2048] per head (head-pairs packed on 128
    parts: head at partition 64*(h%2))
  - v projected seq-major with an appended ones column (v_aug [128,4,65]/chunk)
  - chunked attention, L=128. PSUM accumulation groups must keep a uniform
    operand base partition, so per chunk the work splits by head parity:
    A_even bank = [A(h0) | A(h2) | kT(h0).T | kT(h2).T] (transposes expressed
    as matmul-by-identity so they join the fp32 group), A_odd likewise at
    base 64. One DVE mask per parity -> a_sb bf16; op_even bank accumulates
    a_sb@v_aug + phi_q@S_prev for heads {0,2}, op_odd for {1,3}. KV state S
    [128, 2*130] accumulates in a persistent psum bank across all chunks
    (one long group, base 0); an Act-engine copy snapshots it to SBUF bf16
    each chunk for the next chunk's q@S matmul. The ones column of v_aug
    makes column 64 of each head's op block the normalizer (EPS=1e-6 dropped:
    the denominator is a sum of positive phi products, O(1) or larger).
  - schedule: quarter qt's attention chunks interleave with quarter qt+1's
    projection groups at half-chunk granularity so DVE/Act latency hides
    under projection matmuls and the serial S chain has a full slot of slack.
"""

import threading

import numpy as np

B, S, D, H, HD = 2, 2048, 1024, 16, 64
N_CORES = 8
HPC = 4            # heads per core
HDC = HPC * HD     # 256 projected cols per core
NCHUNK = S // 128  # 16
DC = D // 128      # 8 contraction chunks
NQ = 4             # S quarters
CPQ = NCHUNK // NQ  # chunks per quarter

_lock = threading.Lock()
_cache = {}


def _build_nc():
    import concourse.bass as bass
    import concourse.tile as tile
    from concourse import bacc, mybir

    f32 = mybir.dt.float32
    bf16 = mybir.dt.bfloat16
    Alu = mybir.AluOpType
    Act = mybir.ActivationFunctionType

    nc = bacc.Bacc("TRN2", target_bir_lowering=False, debug=False)

    xqT = nc.dram_tensor("xqT", [D, S], bf16, kind="ExternalInput").ap()
    xkT = nc.dram_tensor("xkT", [D, S], bf16, kind="ExternalInput").ap()
    xvT = nc.dram_tensor("xvT", [D, S], bf16, kind="ExternalInput").ap()
    # host pre-arranged to the SBUF layout [p, dc, m] (p = d % 128)
    wq = nc.dram_tensor("wq", [128, DC, HDC], bf16, kind="ExternalInput").ap()
    wk = nc.dram_tensor("wk", [128, DC, HDC], bf16, kind="ExternalInput").ap()
    wv = nc.dram_tensor("wv", [128, DC, HDC], bf16, kind="ExternalInput").ap()
    out = nc.dram_tensor("out", [S, HDC], f32, kind="ExternalOutput").ap()

    with tile.TileContext(nc) as tc:
        with (
            tc.tile_pool(name="consts", bufs=1) as consts,
            tc.tile_pool(name="weights", bufs=1) as wpool,
            tc.tile_pool(name="resident", bufs=1) as res,
            tc.tile_pool(name="xin", bufs=6) as xin,
            tc.tile_pool(name="work", bufs=3) as work,
            tc.tile_pool(name="attn", bufs=2) as attn,
            tc.tile_pool(name="ps_proj", bufs=2, space="PSUM") as ps_proj,
            tc.tile_pool(name="ps_a", bufs=1, space="PSUM") as ps_a,
            tc.tile_pool(name="ps_op", bufs=1, space="PSUM") as ps_op,
            tc.tile_pool(name="ps_state", bufs=1, space="PSUM") as ps_state,
        ):
            # ---- constants ----
            ones_bf = consts.tile([128, 128], bf16)
            nc.vector.memset(ones_bf[:], 1.0)
            ident = consts.tile([128, 128], bf16)
            nc.gpsimd.affine_select(
                ident[:], ones_bf[:], pattern=[[-1, 128]], base=0,
                channel_multiplier=1, compare_op=Alu.is_equal, fill=0.0,
            )
            ones = consts.tile([128, 256], f32)
            nc.vector.memset(ones[:], 1.0)
            # causal mask in [j (part), head, t (free)] layout: keep j <= t
            maskT2 = consts.tile([128, 2, 128], f32)
            nc.gpsimd.affine_select(
                maskT2[:], ones.rearrange("p (g t) -> p g t", g=2)[:],
                pattern=[[0, 2], [1, 128]], base=0,
                channel_multiplier=-1, compare_op=Alu.is_ge, fill=0.0,
            )

            # ---- weight + input DMAs (SP queue order == issue order) ----
            w_sb = {}
            xt = {}
            w_sb["q"] = wpool.tile([128, DC, HDC], bf16, name="wq_sb")
            nc.sync.dma_start(w_sb["q"][:], wq)

            def load_quarter(xdram, qt, tag):
                t = xin.tile([128, DC, S // NQ], bf16, name=f"x_{tag}_{qt}", tag="xin")
                nc.sync.dma_start(
                    t[:],
                    xdram.rearrange("(dc p) m -> p dc m", p=128)[
                        :, :, qt * (S // NQ):(qt + 1) * (S // NQ)],
                )
                return t

            xt[("q", 0)] = load_quarter(xqT, 0, "q")
            w_sb["k"] = wpool.tile([128, DC, HDC], bf16, name="wk_sb")
            nc.sync.dma_start(w_sb["k"][:], wk)
            xt[("k", 0)] = load_quarter(xkT, 0, "k")
            w_sb["v"] = wpool.tile([128, DC, HDC], bf16, name="wv_sb")
            nc.sync.dma_start(w_sb["v"][:], wv)
            xt[("v", 0)] = load_quarter(xvT, 0, "v")
            for qt in range(1, NQ):
                xt[("q", qt)] = load_quarter(xqT, qt, "q")
                xt[("k", qt)] = load_quarter(xkT, qt, "k")
                xt[("v", qt)] = load_quarter(xvT, qt, "v")

            # ---- resident activations ----
            phi_qT = [res.tile([128, S], bf16, name=f"phi_qT{i}") for i in range(2)]
            phi_kT = [res.tile([128, S], bf16, name=f"phi_kT{i}") for i in range(2)]
            # head index: [hp, par] (head h = 2*hp + par), so parity planes and
            # head-pair blocks are both plain (stride-only) views
            phi_ks = res.tile([128, NCHUNK, 2, 2, 64], bf16, name="phi_ks")
            v_aug = res.tile([128, NCHUNK, HPC, 65], bf16, name="v_aug")
            nc.vector.memset(v_aug[:, :, :, 64:65], 1.0)

            # persistent KV state: head-pair blocks [(g, e)] per hp, base 0
            S_ps = ps_state.tile([128, 2 * 130], f32, name="S_ps")

            def phi_from_psum(ps, dst, n):
                # phi(x) = exp(min(x,0)) + max(x,0); m' = relu(-x); e = exp(-m')
                t1 = work.tile([128, n], f32, tag="phi1")
                nc.scalar.activation(t1[:], ps[:], Act.Relu, scale=-1.0)
                t2 = work.tile([128, n], f32, tag="phi2")
                nc.scalar.activation(t2[:], t1[:], Act.Exp, scale=-1.0)
                nc.vector.scalar_tensor_tensor(
                    dst, ps[:], 0.0, t2[:], op0=Alu.max, op1=Alu.add)

            def proj_units(qt):
                """8 generator units: q-proj hp0/hp1, k-proj hp0/hp1, v c0-c3."""
                sq = S // NQ  # 512
                s0 = qt * sq

                def qk_unit(tname, dst, hp):
                    def emit():
                        x = xt[(tname, qt)]
                        ps = ps_proj.tile([128, sq], f32, tag="proj",
                                          name=f"ps_{tname}_{qt}_{hp}")
                        for dc in range(DC):
                            nc.tensor.matmul(
                                ps[:], w_sb[tname][:, dc, hp * 128:(hp + 1) * 128],
                                x[:, dc, :],
                                start=(dc == 0), stop=(dc == DC - 1),
                            )
                        phi_from_psum(ps, dst[hp][:, s0:s0 + sq], sq)
                    return emit

                def v_unit(cc):
                    def emit():
                        c = qt * CPQ + cc
                        x = xt[("v", qt)]
                        ps = ps_proj.tile([128, HDC], f32, tag="proj",
                                          name=f"ps_v_{c}")
                        for dc in range(DC):
                            nc.tensor.matmul(
                                ps[:], x[:, dc, cc * 128:(cc + 1) * 128],
                                w_sb["v"][:, dc, :],
                                start=(dc == 0), stop=(dc == DC - 1),
                            )
                        nc.scalar.activation(
                            v_aug[:, c, :, 0:64],
                            ps.rearrange("p (h e) -> p h e", h=HPC)[:], Act.Copy)
                    return emit

                units = []
                for tname, dst in (("q", phi_qT), ("k", phi_kT)):
                    for hp in range(2):
                        units.append(qk_unit(tname, dst, hp))
                for cc in range(CPQ):
                    units.append(v_unit(cc))
                return units

            # ---- attention ----
            # per-chunk state shared between front/back halves
            st = {"S_sb": None, "o_sb": None, "a_sb": {}}

            def attn_front(c):
                """A + transpose groups (per parity) and their DVE/Act reads."""
                cs = slice(c * 128, (c + 1) * 128)
                for par in range(2):  # heads (par, par+2), base partition 64*par
                    hb = 64 * par
                    a_ps = ps_a.tile([128, 384], f32, tag=f"A{par}",
                                     name=f"a_ps_{c}_{par}")
                    for i, h in enumerate((par, par + 2)):
                        hp = h // 2
                        nc.tensor.matmul(
                            a_ps[:, i * 128:(i + 1) * 128],
                            phi_kT[hp][hb:hb + 64, cs], phi_qT[hp][hb:hb + 64, cs],
                            start=(i == 0), stop=False,
                        )
                    for i, h in enumerate((par, par + 2)):
                        hp = h // 2
                        nc.tensor.matmul(
                            a_ps[:, 256 + i * 64:256 + (i + 1) * 64],
                            phi_kT[hp][hb:hb + 64, cs], ident[hb:hb + 64, hb:hb + 64],
                            start=False, stop=(i == 1),
                        )
                    # mask (DVE): psum fp32 * maskT2 -> sbuf bf16
                    a_sb = attn.tile([128, 2, 128], bf16, tag=f"Asb{par}",
                                     name=f"a_sb_{c}_{par}")
                    nc.vector.tensor_tensor(
                        a_sb[:], a_ps[:, 0:256].rearrange("p (g t) -> p g t", g=2),
                        maskT2[:], op=Alu.mult)
                    st["a_sb"][(c, par)] = a_sb
                    # phi_k seq-major: psum fp32 -> sbuf bf16, parity plane
                    phks_dst = phi_ks[:, c, :, par, :]
                    phks_src = a_ps[:, 256:384].rearrange("p (i e) -> p i e", i=2)
                    if par == 0:
                        nc.scalar.activation(phks_dst, phks_src, Act.Copy)
                    else:
                        nc.vector.tensor_copy(phks_dst, phks_src)

            def attn_back(c):
                cs = slice(c * 128, (c + 1) * 128)
                S_sb = st["S_sb"]
                op_ps = {}
                for par in range(2):
                    hb = 64 * par
                    op = ps_op.tile([128, 130], f32, tag=f"op{par}",
                                    name=f"op_ps_{c}_{par}")
                    a_sb = st["a_sb"].pop((c, par))
                    for i, h in enumerate((par, par + 2)):
                        hp = h // 2
                        nc.tensor.matmul(
                            op[:, i * 65:(i + 1) * 65],
                            a_sb[:, i, :], v_aug[:, c, h, :],
                            start=(i == 0), stop=(c == 0 and i == 1),
                        )
                        if c > 0:
                            nc.tensor.matmul(
                                op[:, i * 65:(i + 1) * 65],
                                phi_qT[hp][hb:hb + 64, cs],
                                S_sb[hb:hb + 64, 130 * hp + 65 * par:
                                     130 * hp + 65 * par + 65],
                                start=False, stop=(i == 1),
                            )
                    op_ps[par] = op
                # state increment for chunk c (PE, one long group, base 0)
                for hp in range(2):
                    nc.tensor.matmul(
                        S_ps[:, 130 * hp:130 * (hp + 1)],
                        phi_ks[:, c, hp].rearrange("p a e -> p (a e)"),
                        v_aug[:, c, 2 * hp:2 * hp + 2, :],
                        start=(c == 0 and hp == 0),
                        stop=(c == NCHUNK - 1 and hp == 1),
                    )
                # snapshot state for chunk c+1 (Act): psum fp32 -> sbuf bf16
                if c < NCHUNK - 1:
                    S_new = attn.tile([128, 2 * 130], bf16, tag="Ssb",
                                      name=f"S_sb_{c}")
                    nc.scalar.activation(S_new[:], S_ps[:], Act.Copy)
                    st["S_sb"] = S_new
                # normalize (DVE): rcp of the ones-column, then scale
                qt, cc = c // CPQ, c % CPQ
                if cc == 0:
                    st["o_sb"] = work.tile([128, CPQ, 2, 2, 64], f32, tag="osb",
                                           name=f"o_sb_{qt}", bufs=2)
                o_sb = st["o_sb"]
                for par in range(2):
                    op4 = op_ps[par].rearrange("p (i e) -> p i e", i=2)
                    rcp = attn.tile([128, 2], f32, tag=f"rcp{par}",
                                    name=f"rcp_{c}_{par}")
                    nc.vector.reciprocal(rcp[:], op4[:, :, 64])
                    nc.vector.tensor_tensor(
                        o_sb[:, cc, :, par, :], op4[:, :, 0:64],
                        rcp[:].broadcast_to([128, 2, 64]),
                        op=Alu.mult)
                if cc == CPQ - 1:
                    nc.sync.dma_start(
                        out.rearrange("(q c p) he -> q p c he", c=CPQ, p=128)[qt],
                        o_sb.rearrange("p c a b e -> p c (a b e)")[:])

            # ---- global schedule ----
            # quarter qt's attention interleaves with quarter qt+1's projection
            # units at half-chunk granularity.
            for u in proj_units(0):
                u()
            for qt in range(1, NQ):
                units = proj_units(qt)
                for i, u in enumerate(units):
                    u()
                    c = (qt - 1) * CPQ + i // 2
                    if i % 2 == 0:
                        attn_front(c)
                    else:
                        attn_back(c)
            # tail quarter: one-chunk software pipeline skew
            c0 = (NQ - 1) * CPQ
            attn_front(c0)
            for cc in range(1, CPQ):
                attn_front(c0 + cc)
                attn_back(c0 + cc - 1)
            attn_back(c0 + CPQ - 1)

    nc.compile()
    return nc


def _get_nc():
    with _lock:
        if "nc" not in _cache:
            _cache["nc"] = _build_nc()
        return _cache["nc"]


def kernel(query, key, value, query_kernel, key_kernel, value_kernel):
    import ml_dtypes
    from concourse.bass_utils import run_bass_kernel_spmd

    nc = _get_nc()
    bf16 = ml_dtypes.bfloat16

    xT = {}
    for b in range(B):
        xT[("q", b)] = np.ascontiguousarray(query[b].T.astype(bf16))
        xT[("k", b)] = np.ascontiguousarray(key[b].T.astype(bf16))
        xT[("v", b)] = np.ascontiguousarray(value[b].T.astype(bf16))

    def w_arrange(wk_full, h0):
        w = wk_full[:, h0:h0 + HPC, :].reshape(D, HDC)  # [D, HDC]
        # [p, dc, m] with p = d % 128, dc = d // 128
        return np.ascontiguousarray(
            w.reshape(DC, 128, HDC).transpose(1, 0, 2).astype(bf16))

    in_maps = []
    for c in range(N_CORES):
        b, h0 = c // 4, 4 * (c % 4)
        in_maps.append({
            "xqT": xT[("q", b)],
            "xkT": xT[("k", b)],
            "xvT": xT[("v", b)],
            "wq": w_arrange(query_kernel, h0),
            "wk": w_arrange(key_kernel, h0),
            "wv": w_arrange(value_kernel, h0),
        })

    results = run_bass_kernel_spmd(nc, in_maps, core_ids=list(range(N_CORES)))

    # The reference ends with a FLAT reshape of [B*H, S, HD] -> (B, S, H*HD):
    # output rows [128h:128h+128] of batch b are head h's [S, HD] attention
    # output flat-reshaped to [128, H*HD].
    full = np.empty((B, S, H * HD), dtype=np.float32)
    for c in range(N_CORES):
        b, h0 = c // 4, 4 * (c % 4)
        av = np.asarray(results.results[c]["out"], dtype=np.float32).reshape(S, HPC, HD)
        for hl in range(HPC):
            full[b, (h0 + hl) * 128:(h0 + hl + 1) * 128, :] = (
                av[:, hl, :].reshape(128, H * HD))
    return full


# revision 24
# speedup vs baseline: 1.5913x; 1.0974x over previous
"""Causal linear attention (elu+1 feature map) for Trainium2, 8 NeuronCores.

Problem: B=2, S=2048, D=1024, H=16, HD=64.
  q/k/v projections [S,D]@[D,H*HD], phi = elu+1, causal linear attention
  out[t] = (sum_{i<=t} (phi_q[t].phi_k[i]) v[i]) / (phi_q[t].sum_{i<=t} phi_k[i] + eps)

Sharding: core c -> (batch b=c//4, heads h0=4*(c%4) .. h0+3). No cross-core comm.
Host feeds x^T [D,S] per core in bf16 so the contraction dim d sits on SBUF
partitions with no on-chip transposes, and DMA bytes are halved.

Device algorithm (per core, 4 heads, bf16 matmuls / fp32 psum):
  - proj q,k -> phi_qT/phi_kT [64,2048] per head (head-pairs packed on 128
    parts: head at partition 64*(h%2))
  - phi_k seq-major (phi_ks) via XBAR DMA block transposes: one
    dma_start_transpose per (quarter, head-pair) turns phi_kT [128, 4*128]
    into [128 s, 4 chunk, 128 d] directly in SBUF - no PE transposes, no
    psum->SBUF copies. Triggered from the Activation queue so the SP queue
    keeps streaming x prefetches.
  - v projected seq-major with an appended ones column (v_aug [128,4,65]/chunk)
  - chunked attention, L=128. PSUM accumulation groups must keep a uniform
    operand base partition, so per chunk the work splits by head parity
    (even heads {0,2} read partitions 0:64 of the phi tiles, odd {1,3} read
    64:128). Per (chunk, parity) ONE psum bank holds [A(h) | A(h+2) | op]:
    the group opens with the two A matmuls, a DVE mask turns A into a_sb
    bf16, then the op matmuls (a_sb@v_aug + phi_q@S_prev per head) continue
    the same group and close it. KV state S [128, 2*130] accumulates in a
    persistent psum bank across all chunks (one long group, base 0); an Act
    copy snapshots it to SBUF bf16 each chunk for the next chunk's q@S
    matmul. The ones column of v_aug makes column 64 of each head's op block
    the normalizer (EPS=1e-6 dropped: the denominator is a sum of positive
    phi products, O(1) or larger).
  - schedule: attention chunks run inside their own quarter, interleaved with
    the v-projection units, with a one-slot front/back software pipeline so
    every DVE/Act hop hides under PE matmuls. Only the last chunk's back
    half trails the final projection.
"""

import threading

import numpy as np

B, S, D, H, HD = 2, 2048, 1024, 16, 64
N_CORES = 8
HPC = 4            # heads per core
HDC = HPC * HD     # 256 projected cols per core
NCHUNK = S // 128  # 16
DC = D // 128      # 8 contraction chunks
NQ = 4             # S quarters
CPQ = NCHUNK // NQ  # chunks per quarter

_lock = threading.Lock()
_cache = {}


def _build_nc():
    import concourse.bass as bass
    import concourse.tile as tile
    from concourse import bacc, mybir

    f32 = mybir.dt.float32
    bf16 = mybir.dt.bfloat16
    Alu = mybir.AluOpType
    Act = mybir.ActivationFunctionType

    nc = bacc.Bacc("TRN2", target_bir_lowering=False, debug=False)

    xqT = nc.dram_tensor("xqT", [D, S], bf16, kind="ExternalInput").ap()
    xkT = nc.dram_tensor("xkT", [D, S], bf16, kind="ExternalInput").ap()
    xvT = nc.dram_tensor("xvT", [D, S], bf16, kind="ExternalInput").ap()
    # host pre-arranged to the SBUF layout [p, dc, m] (p = d % 128)
    wq = nc.dram_tensor("wq", [128, DC, HDC], bf16, kind="ExternalInput").ap()
    wk = nc.dram_tensor("wk", [128, DC, HDC], bf16, kind="ExternalInput").ap()
    wv = nc.dram_tensor("wv", [128, DC, HDC], bf16, kind="ExternalInput").ap()
    out = nc.dram_tensor("out", [S, HDC], bf16, kind="ExternalOutput").ap()

    with tile.TileContext(nc) as tc:
        with (
            tc.tile_pool(name="consts", bufs=1) as consts,
            tc.tile_pool(name="weights", bufs=1) as wpool,
            tc.tile_pool(name="resident", bufs=1) as res,
            tc.tile_pool(name="xin", bufs=6) as xin,
            tc.tile_pool(name="work", bufs=3) as work,
            tc.tile_pool(name="attn", bufs=2) as attn,
            tc.tile_pool(name="ps_proj", bufs=3, space="PSUM") as ps_proj,
            tc.tile_pool(name="ps_a", bufs=1, space="PSUM") as ps_a,
            tc.tile_pool(name="ps_op", bufs=1, space="PSUM") as ps_op,
            tc.tile_pool(name="ps_state", bufs=1, space="PSUM") as ps_state,
        ):
            # ---- constants ----
            ones_bf = consts.tile([128, 128], bf16)
            nc.vector.memset(ones_bf[:], 1.0)
            ident = consts.tile([128, 128], bf16)
            nc.gpsimd.affine_select(
                ident[:], ones_bf[:], pattern=[[-1, 128]], base=0,
                channel_multiplier=1, compare_op=Alu.is_equal, fill=0.0,
            )
            ones = consts.tile([128, 256], f32)
            nc.vector.memset(ones[:], 1.0)
            # causal mask in [j (part), head, t (free)] layout: keep j <= t
            maskT2 = consts.tile([128, 2, 128], f32)
            nc.gpsimd.affine_select(
                maskT2[:], ones.rearrange("p (g t) -> p g t", g=2)[:],
                pattern=[[0, 2], [1, 128]], base=0,
                channel_multiplier=-1, compare_op=Alu.is_ge, fill=0.0,
            )

            # ---- weight + input DMAs (SP queue order == issue order) ----
            # startup fast path: halved wq/xq0/wk/xk0 DMAs so the first
            # projection matmuls start ~2.2us in instead of ~4.4us
            w_sb = {}
            xt = {}

            def load_quarter(xdram, qt, tag, split=False):
                t = xin.tile([128, DC, S // NQ], bf16, name=f"x_{tag}_{qt}", tag="xin")
                src = xdram.rearrange("(dc p) m -> p dc m", p=128)[
                    :, :, qt * (S // NQ):(qt + 1) * (S // NQ)]
                if split:
                    return t, (lambda: nc.sync.dma_start(t[:, 0:DC // 2], src[:, 0:DC // 2]),
                               lambda: nc.sync.dma_start(t[:, DC // 2:], src[:, DC // 2:]))
                nc.sync.dma_start(t[:], src)
                return t

            for name, wdram in (("q", wq), ("k", wk), ("v", wv)):
                w_sb[name] = wpool.tile([128, DC, HDC], bf16, name=f"w{name}_sb")
            for name, wdram, xdram in (("q", wq, xqT), ("k", wk, xkT)):
                nc.sync.dma_start(w_sb[name][:, :, 0:128], wdram[:, :, 0:128])
                t, (dma_a, dma_b) = load_quarter(xdram, 0, name, split=True)
                dma_a()
                nc.sync.dma_start(w_sb[name][:, :, 128:256], wdram[:, :, 128:256])
                dma_b()
                xt[(name, 0)] = t
            nc.sync.dma_start(w_sb["v"][:], wv)
            # xv0 in S-halves: the first v-proj chunks start ~1.4us earlier
            xv0 = xin.tile([128, DC, S // NQ], bf16, name="x_v_0", tag="xin")
            xv0_src = xvT.rearrange("(dc p) m -> p dc m", p=128)[:, :, 0:S // NQ]
            nc.sync.dma_start(xv0[:, :, 0:256], xv0_src[:, :, 0:256])
            nc.sync.dma_start(xv0[:, :, 256:512], xv0_src[:, :, 256:512])
            xt[("v", 0)] = xv0
            for qt in range(1, NQ):
                xt[("q", qt)] = load_quarter(xqT, qt, "q")
                xt[("k", qt)] = load_quarter(xkT, qt, "k")
                xt[("v", qt)] = load_quarter(xvT, qt, "v")

            # ---- resident activations ----
            phi_qT = [res.tile([128, S], bf16, name=f"phi_qT{i}") for i in range(2)]
            phi_kT = [res.tile([128, S], bf16, name=f"phi_kT{i}") for i in range(2)]
            # seq-major phi_k: [s, chunk, hp, par, e] (head h = 2*hp + par)
            phi_ks = res.tile([128, NCHUNK, 2, 2, 64], bf16, name="phi_ks")
            v_aug = res.tile([128, NCHUNK, HPC, 65], bf16, name="v_aug")
            nc.vector.memset(v_aug[:, :, :, 64:65], 1.0)

            # persistent KV state: head-pair blocks [(g, e)] per hp, base 0
            S_ps = ps_state.tile([128, 2 * 130], f32, name="S_ps")

            def phi_from_psum(ps, dst, n):
                # phi(x) = exp(min(x,0)) + max(x,0); m' = relu(-x); e = exp(-m')
                t1 = work.tile([128, n], f32, tag="phi1")
                nc.scalar.activation(t1[:], ps[:], Act.Relu, scale=-1.0)
                t2 = work.tile([128, n], f32, tag="phi2")
                nc.scalar.activation(t2[:], t1[:], Act.Exp, scale=-1.0)
                nc.vector.scalar_tensor_tensor(
                    dst, ps[:], 0.0, t2[:], op0=Alu.max, op1=Alu.add)

            def qk_unit(qt, tname, dst, hp):
                sq = S // NQ
                s0 = qt * sq

                def emit():
                    x = xt[(tname, qt)]
                    ps = ps_proj.tile([128, sq], f32, tag="proj",
                                      name=f"ps_{tname}_{qt}_{hp}")
                    for dc in range(DC):
                        nc.tensor.matmul(
                            ps[:], w_sb[tname][:, dc, hp * 128:(hp + 1) * 128],
                            x[:, dc, :],
                            start=(dc == 0), stop=(dc == DC - 1),
                        )
                    phi_from_psum(ps, dst[hp][:, s0:s0 + sq], sq)
                return emit

            def v_unit(qt, cc):
                def emit():
                    c = qt * CPQ + cc
                    x = xt[("v", qt)]
                    ps = ps_proj.tile([128, HDC], f32, tag="proj", name=f"ps_v_{c}")
                    for dc in range(DC):
                        nc.tensor.matmul(
                            ps[:], x[:, dc, cc * 128:(cc + 1) * 128],
                            w_sb["v"][:, dc, :],
                            start=(dc == 0), stop=(dc == DC - 1),
                        )
                    nc.scalar.activation(
                        v_aug[:, c, :, 0:64],
                        ps.rearrange("p (h e) -> p h e", h=HPC)[:], Act.Copy)
                return emit

            # ---- attention ----
            st = {"S_sb": None, "o_sb": None, "aop": {}}

            def attn_front(c):
                """A + transpose groups (per parity); mask on DVE; phi_ks copy."""
                cs = slice(c * 128, (c + 1) * 128)
                for par in range(2):
                    hb = 64 * par
                    a_ps = ps_a.tile([128, 384], f32, tag=f"A{par}",
                                     name=f"a_ps_{c}_{par}")
                    for i, h in enumerate((par, par + 2)):
                        hp = h // 2
                        nc.tensor.matmul(
                            a_ps[:, i * 128:(i + 1) * 128],
                            phi_kT[hp][hb:hb + 64, cs], phi_qT[hp][hb:hb + 64, cs],
                            start=(i == 0), stop=False,
                        )
                    for i, h in enumerate((par, par + 2)):
                        hp = h // 2
                        nc.tensor.matmul(
                            a_ps[:, 256 + i * 64:256 + (i + 1) * 64],
                            phi_kT[hp][hb:hb + 64, cs], ident[hb:hb + 64, hb:hb + 64],
                            start=False, stop=(i == 1),
                        )
                    a_sb = attn.tile([128, 2, 128], bf16, tag=f"Asb{par}",
                                     name=f"a_sb_{c}_{par}")
                    nc.vector.tensor_tensor(
                        a_sb[:], a_ps[:, 0:256].rearrange("p (g t) -> p g t", g=2),
                        maskT2[:], op=Alu.mult)
                    # phi_k seq-major: psum fp32 -> sbuf bf16, parity plane
                    # (Act engine both times: DVE is the binding engine in the
                    # attention-dense stretch)
                    nc.scalar.activation(
                        phi_ks[:, c, :, par, :],
                        a_ps[:, 256:384].rearrange("p (i e) -> p i e", i=2),
                        Act.Copy)
                    st["aop"][(c, par)] = a_sb

            def attn_back(c):
                cs = slice(c * 128, (c + 1) * 128)
                S_sb = st["S_sb"]
                op_ps = {}
                for par in range(2):
                    hb = 64 * par
                    a_sb = st["aop"].pop((c, par))
                    op = ps_op.tile([128, 130], f32, tag=f"op{par}",
                                    name=f"op_ps_{c}_{par}")
                    for i, h in enumerate((par, par + 2)):
                        hp = h // 2
                        nc.tensor.matmul(
                            op[:, i * 65:(i + 1) * 65],
                            a_sb[:, i, :], v_aug[:, c, h, :],
                            start=(i == 0), stop=(c == 0 and i == 1),
                        )
                        if c > 0:
                            nc.tensor.matmul(
                                op[:, i * 65:(i + 1) * 65],
                                phi_qT[hp][hb:hb + 64, cs],
                                S_sb[hb:hb + 64, 130 * hp + 65 * par:
                                     130 * hp + 65 * par + 65],
                                start=False, stop=(i == 1),
                            )
                    op_ps[par] = op
                # state increment for chunk c (PE, one long group, base 0)
                for hp in range(2):
                    nc.tensor.matmul(
                        S_ps[:, 130 * hp:130 * (hp + 1)],
                        phi_ks[:, c, hp].rearrange("p a e -> p (a e)"),
                        v_aug[:, c, 2 * hp:2 * hp + 2, :],
                        start=(c == 0 and hp == 0),
                        stop=(c == NCHUNK - 1 and hp == 1),
                    )
                # snapshot state for chunk c+1 (Act): psum fp32 -> sbuf bf16
                if c < NCHUNK - 1:
                    S_new = attn.tile([128, 2 * 130], bf16, tag="Ssb",
                                      name=f"S_sb_{c}")
                    nc.scalar.activation(S_new[:], S_ps[:], Act.Copy)
                    st["S_sb"] = S_new
                # normalize (DVE): rcp of the ones-column, then scale
                qt, cc = c // CPQ, c % CPQ
                if cc == 0:
                    st["o_sb"] = work.tile([128, CPQ, 2, 2, 64], bf16, tag="osb",
                                           name=f"o_sb_{qt}", bufs=2)
                o_sb = st["o_sb"]
                for par in range(2):
                    op4 = op_ps[par].rearrange("p (i e) -> p i e", i=2)
                    rcp = attn.tile([128, 2], f32, tag=f"rcp{par}",
                                    name=f"rcp_{c}_{par}")
                    nc.vector.reciprocal(rcp[:], op4[:, :, 64])
                    nc.vector.tensor_tensor(
                        o_sb[:, cc, :, par, :], op4[:, :, 0:64],
                        rcp[:].broadcast_to([128, 2, 64]),
                        op=Alu.mult)
                out_q = out.rearrange("(q c p) he -> q p c he", c=CPQ, p=128)[qt]
                if qt == NQ - 1:
                    # tail: per-chunk stores, alternating trigger queues so the
                    # HWDGE/DGE latencies of consecutive stores overlap
                    eng = nc.sync if cc % 2 == 0 else nc.scalar
                    eng.dma_start(
                        out_q[:, cc:cc + 1, :],
                        o_sb.rearrange("p c a b e -> p c (a b e)")[:, cc:cc + 1, :])
                elif cc == CPQ - 1:
                    nc.sync.dma_start(
                        out_q, o_sb.rearrange("p c a b e -> p c (a b e)")[:])

            # ---- global schedule ----
            # qt0: projections only (DMA-starved anyway). qt1/qt2: previous
            # quarter's 4 chunks at half-unit pace. qt3: chunks 8-15 (quarter
            # 2's and its own) at one per unit with a one-slot F/B skew, so
            # only chunk 15's back half trails the final projection.
            def units_of(qt):
                return [qk_unit(qt, "q", phi_qT, 0), qk_unit(qt, "q", phi_qT, 1),
                        qk_unit(qt, "k", phi_kT, 0), qk_unit(qt, "k", phi_kT, 1),
                        v_unit(qt, 0), v_unit(qt, 1), v_unit(qt, 2), v_unit(qt, 3)]

            for u in units_of(0):
                u()
            for qt in (1, 2):
                # frontloaded: all four chunks' work lands in the first five
                # units, exactly where the x-prefetch DMAs starve the PE
                c0 = (qt - 1) * CPQ
                plan = {0: [("F", 0)], 1: [("F", 1), ("B", 0)],
                        2: [("F", 2), ("B", 1)], 3: [("F", 3), ("B", 2)],
                        4: [("B", 3)]}
                for i, u in enumerate(units_of(qt)):
                    u()
                    for kind, cc in plan.get(i, []):
                        (attn_front if kind == "F" else attn_back)(c0 + cc)
            for i, u in enumerate(units_of(3)):
                u()
                attn_front(8 + i)
                if i > 0:
                    attn_back(8 + i - 1)
            attn_back(15)

    nc.compile()
    return nc


def _get_nc():
    with _lock:
        if "nc" not in _cache:
            _cache["nc"] = _build_nc()
        return _cache["nc"]


def kernel(query, key, value, query_kernel, key_kernel, value_kernel):
    import ml_dtypes
    from concourse.bass_utils import run_bass_kernel_spmd

    nc = _get_nc()
    bf16 = ml_dtypes.bfloat16

    xT = {}
    for b in range(B):
        xT[("q", b)] = np.ascontiguousarray(query[b].T.astype(bf16))
        xT[("k", b)] = np.ascontiguousarray(key[b].T.astype(bf16))
        xT[("v", b)] = np.ascontiguousarray(value[b].T.astype(bf16))

    def w_arrange(wk_full, h0):
        w = wk_full[:, h0:h0 + HPC, :].reshape(D, HDC)  # [D, HDC]
        # [p, dc, m] with p = d % 128, dc = d // 128
        return np.ascontiguousarray(
            w.reshape(DC, 128, HDC).transpose(1, 0, 2).astype(bf16))

    in_maps = []
    for c in range(N_CORES):
        b, h0 = c // 4, 4 * (c % 4)
        in_maps.append({
            "xqT": xT[("q", b)],
            "xkT": xT[("k", b)],
            "xvT": xT[("v", b)],
            "wq": w_arrange(query_kernel, h0),
            "wk": w_arrange(key_kernel, h0),
            "wv": w_arrange(value_kernel, h0),
        })

    results = run_bass_kernel_spmd(nc, in_maps, core_ids=list(range(N_CORES)))

    # The reference ends with a FLAT reshape of [B*H, S, HD] -> (B, S, H*HD):
    # output rows [128h:128h+128] of batch b are head h's [S, HD] attention
    # output flat-reshaped to [128, H*HD].
    full = np.empty((B, S, H * HD), dtype=np.float32)
    for c in range(N_CORES):
        b, h0 = c // 4, 4 * (c % 4)
        av = np.asarray(results.results[c]["out"]).astype(np.float32).reshape(S, HPC, HD)
        for hl in range(HPC):
            full[b, (h0 + hl) * 128:(h0 + hl + 1) * 128, :] = (
                av[:, hl, :].reshape(128, H * HD))
    return full


# revision 40
# speedup vs baseline: 1.6278x; 1.0230x over previous
"""Causal linear attention (elu+1 feature map) for Trainium2, 8 NeuronCores.

Problem: B=2, S=2048, D=1024, H=16, HD=64.
  q/k/v projections [S,D]@[D,H*HD], phi = elu+1, causal linear attention
  out[t] = (sum_{i<=t} (phi_q[t].phi_k[i]) v[i]) / (phi_q[t].sum_{i<=t} phi_k[i] + eps)

Sharding: core c -> (batch b=c//4, heads h0=4*(c%4) .. h0+3). No cross-core comm.
Host feeds x^T [D,S] per core in bf16 so the contraction dim d sits on SBUF
partitions with no on-chip transposes, and DMA bytes are halved.

Device algorithm (per core, 4 heads, bf16 matmuls / fp32 psum):
  - proj q,k -> phi_qT/phi_kT [64,2048] per head (head-pairs packed on 128
    parts: head at partition 64*(h%2))
  - phi_k seq-major (phi_ks) via XBAR DMA block transposes: one
    dma_start_transpose per (quarter, head-pair) turns phi_kT [128, 4*128]
    into [128 s, 4 chunk, 128 d] directly in SBUF - no PE transposes, no
    psum->SBUF copies. Triggered from the Activation queue so the SP queue
    keeps streaming x prefetches.
  - v projected seq-major with an appended ones column (v_aug [128,4,65]/chunk)
  - chunked attention, L=128. PSUM accumulation groups must keep a uniform
    operand base partition, so per chunk the work splits by head parity
    (even heads {0,2} read partitions 0:64 of the phi tiles, odd {1,3} read
    64:128). Per (chunk, parity) ONE psum bank holds [A(h) | A(h+2) | op]:
    the group opens with the two A matmuls, a DVE mask turns A into a_sb
    bf16, then the op matmuls (a_sb@v_aug + phi_q@S_prev per head) continue
    the same group and close it. KV state S [128, 2*130] accumulates in a
    persistent psum bank across all chunks (one long group, base 0); an Act
    copy snapshots it to SBUF bf16 each chunk for the next chunk's q@S
    matmul. The ones column of v_aug makes column 64 of each head's op block
    the normalizer (EPS=1e-6 dropped: the denominator is a sum of positive
    phi products, O(1) or larger).
  - schedule: attention chunks run inside their own quarter, interleaved with
    the v-projection units, with a one-slot front/back software pipeline so
    every DVE/Act hop hides under PE matmuls. Only the last chunk's back
    half trails the final projection.
"""

import threading

import numpy as np

B, S, D, H, HD = 2, 2048, 1024, 16, 64
N_CORES = 8
HPC = 4            # heads per core
HDC = HPC * HD     # 256 projected cols per core
NCHUNK = S // 128  # 16
DC = D // 128      # 8 contraction chunks
NQ = 4             # S quarters
CPQ = NCHUNK // NQ  # chunks per quarter

_lock = threading.Lock()
_cache = {}


def _build_nc():
    import concourse.bass as bass
    import concourse.tile as tile
    from concourse import bacc, mybir

    f32 = mybir.dt.float32
    bf16 = mybir.dt.bfloat16
    Alu = mybir.AluOpType
    Act = mybir.ActivationFunctionType

    nc = bacc.Bacc("TRN2", target_bir_lowering=False, debug=False)

    xqT = nc.dram_tensor("xqT", [D, S], bf16, kind="ExternalInput").ap()
    xkT = nc.dram_tensor("xkT", [D, S], bf16, kind="ExternalInput").ap()
    xvT = nc.dram_tensor("xvT", [D, S], bf16, kind="ExternalInput").ap()
    # host pre-arranged to the SBUF layout [p, dc, m] (p = d % 128)
    wq = nc.dram_tensor("wq", [128, DC, HDC], bf16, kind="ExternalInput").ap()
    wk = nc.dram_tensor("wk", [128, DC, HDC], bf16, kind="ExternalInput").ap()
    wv = nc.dram_tensor("wv", [128, DC, HDC], bf16, kind="ExternalInput").ap()
    out = nc.dram_tensor("out", [S, HDC], bf16, kind="ExternalOutput").ap()

    with tile.TileContext(nc) as tc:
        with (
            tc.tile_pool(name="consts", bufs=1) as consts,
            tc.tile_pool(name="weights", bufs=1) as wpool,
            tc.tile_pool(name="resident", bufs=1) as res,
            tc.tile_pool(name="xin", bufs=6) as xin,
            tc.tile_pool(name="work", bufs=3) as work,
            tc.tile_pool(name="attn", bufs=2) as attn,
            tc.tile_pool(name="ps_proj", bufs=3, space="PSUM") as ps_proj,
            tc.tile_pool(name="ps_a", bufs=1, space="PSUM") as ps_a,
            tc.tile_pool(name="ps_op", bufs=1, space="PSUM") as ps_op,
            tc.tile_pool(name="ps_state", bufs=1, space="PSUM") as ps_state,
        ):
            # ---- constants ----
            ones_bf = consts.tile([128, 128], bf16)
            nc.vector.memset(ones_bf[:], 1.0)
            ident = consts.tile([128, 128], bf16)
            nc.gpsimd.affine_select(
                ident[:], ones_bf[:], pattern=[[-1, 128]], base=0,
                channel_multiplier=1, compare_op=Alu.is_equal, fill=0.0,
            )
            ones = consts.tile([128, 256], f32)
            nc.vector.memset(ones[:], 1.0)
            # causal mask in [j (part), head, t (free)] layout: keep j <= t
            maskT2 = consts.tile([128, 2, 128], f32)
            nc.gpsimd.affine_select(
                maskT2[:], ones.rearrange("p (g t) -> p g t", g=2)[:],
                pattern=[[0, 2], [1, 128]], base=0,
                channel_multiplier=-1, compare_op=Alu.is_ge, fill=0.0,
            )

            # ---- weight + input DMAs (SP queue order == issue order) ----
            # startup fast path: halved wq/xq0/wk/xk0 DMAs so the first
            # projection matmuls start ~2.2us in instead of ~4.4us
            w_sb = {}
            xt = {}

            def load_quarter(xdram, qt, tag, split=False):
                t = xin.tile([128, DC, S // NQ], bf16, name=f"x_{tag}_{qt}", tag="xin")
                src = xdram.rearrange("(dc p) m -> p dc m", p=128)[
                    :, :, qt * (S // NQ):(qt + 1) * (S // NQ)]
                if split:
                    return t, (lambda: nc.sync.dma_start(t[:, 0:DC // 2], src[:, 0:DC // 2]),
                               lambda: nc.sync.dma_start(t[:, DC // 2:], src[:, DC // 2:]))
                nc.sync.dma_start(t[:], src)
                return t

            for name, wdram in (("q", wq), ("k", wk), ("v", wv)):
                w_sb[name] = wpool.tile([128, DC, HDC], bf16, name=f"w{name}_sb")
            for name, wdram, xdram in (("q", wq, xqT), ("k", wk, xkT)):
                nc.sync.dma_start(w_sb[name][:, :, 0:128], wdram[:, :, 0:128])
                t, (dma_a, dma_b) = load_quarter(xdram, 0, name, split=True)
                dma_a()
                nc.sync.dma_start(w_sb[name][:, :, 128:256], wdram[:, :, 128:256])
                dma_b()
                xt[(name, 0)] = t
            nc.sync.dma_start(w_sb["v"][:], wv)
            # xv0 in S-halves: the first v-proj chunks start ~1.4us earlier
            xv0 = xin.tile([128, DC, S // NQ], bf16, name="x_v_0", tag="xin")
            xv0_src = xvT.rearrange("(dc p) m -> p dc m", p=128)[:, :, 0:S // NQ]
            nc.sync.dma_start(xv0[:, :, 0:256], xv0_src[:, :, 0:256])
            nc.sync.dma_start(xv0[:, :, 256:512], xv0_src[:, :, 256:512])
            xt[("v", 0)] = xv0
            for qt in range(1, NQ):
                xt[("q", qt)] = load_quarter(xqT, qt, "q")
                xt[("k", qt)] = load_quarter(xkT, qt, "k")
                xt[("v", qt)] = load_quarter(xvT, qt, "v")

            # ---- resident activations ----
            phi_qT = [res.tile([128, S], bf16, name=f"phi_qT{i}") for i in range(2)]
            phi_kT = [res.tile([128, S], bf16, name=f"phi_kT{i}") for i in range(2)]
            # seq-major phi_k: [s, chunk, hp, par, e] (head h = 2*hp + par)
            phi_ks = res.tile([128, NCHUNK, 2, 2, 64], bf16, name="phi_ks")
            v_aug = res.tile([128, NCHUNK, HPC, 65], bf16, name="v_aug")
            nc.vector.memset(v_aug[:, :, :, 64:65], 1.0)

            # persistent KV state: head-pair blocks [(g, e)] per hp, base 0
            S_ps = ps_state.tile([128, 2 * 130], f32, name="S_ps")

            # dummy matmuls into the (not-yet-started) state bank: keep the PE
            # p-state ramp alive across known DMA-wait gaps in the startup
            # phase. s_inc(0)'s start=True zeroes the bank afterwards.
            warm_budget = [True]

            def warm(n):
                if not warm_budget[0]:
                    return
                for _ in range(n):
                    nc.tensor.matmul(S_ps[:, 0:128], ones_bf[:], ones_bf[:],
                                     start=True, stop=True)

            def phi_from_psum(ps, dst, n):
                # phi(x) = exp(min(x,0)) + max(x,0); m' = relu(-x); e = exp(-m')
                t1 = work.tile([128, n], f32, tag="phi1")
                nc.scalar.activation(t1[:], ps[:], Act.Relu, scale=-1.0)
                t2 = work.tile([128, n], f32, tag="phi2")
                nc.scalar.activation(t2[:], t1[:], Act.Exp, scale=-1.0)
                nc.vector.scalar_tensor_tensor(
                    dst, ps[:], 0.0, t2[:], op0=Alu.max, op1=Alu.add)

            def qk_unit(qt, tname, dst, hp):
                sq = S // NQ
                s0 = qt * sq

                def emit():
                    x = xt[(tname, qt)]
                    ps = ps_proj.tile([128, sq], f32, tag="proj",
                                      name=f"ps_{tname}_{qt}_{hp}")
                    for dc in range(DC):
                        nc.tensor.matmul(
                            ps[:], w_sb[tname][:, dc, hp * 128:(hp + 1) * 128],
                            x[:, dc, :],
                            start=(dc == 0), stop=(dc == DC - 1),
                        )
                    phi_from_psum(ps, dst[hp][:, s0:s0 + sq], sq)
                return emit

            def v_unit(qt, cc):
                def emit():
                    c = qt * CPQ + cc
                    x = xt[("v", qt)]
                    ps = ps_proj.tile([128, HDC], f32, tag="proj", name=f"ps_v_{c}")
                    for dc in range(DC):
                        nc.tensor.matmul(
                            ps[:], x[:, dc, cc * 128:(cc + 1) * 128],
                            w_sb["v"][:, dc, :],
                            start=(dc == 0), stop=(dc == DC - 1),
                        )
                    nc.scalar.activation(
                        v_aug[:, c, :, 0:64],
                        ps.rearrange("p (h e) -> p h e", h=HPC)[:], Act.Copy)
                return emit

            # ---- attention ----
            st = {"S_sb": None, "o_sb": None, "aop": {}}

            def attn_front(c):
                """A + transpose groups (per parity); mask on DVE; phi_ks copy."""
                cs = slice(c * 128, (c + 1) * 128)
                for par in range(2):
                    hb = 64 * par
                    a_ps = ps_a.tile([128, 384], f32, tag=f"A{par}",
                                     name=f"a_ps_{c}_{par}")
                    for i, h in enumerate((par, par + 2)):
                        hp = h // 2
                        nc.tensor.matmul(
                            a_ps[:, i * 128:(i + 1) * 128],
                            phi_kT[hp][hb:hb + 64, cs], phi_qT[hp][hb:hb + 64, cs],
                            start=(i == 0), stop=False,
                        )
                    for i, h in enumerate((par, par + 2)):
                        hp = h // 2
                        nc.tensor.matmul(
                            a_ps[:, 256 + i * 64:256 + (i + 1) * 64],
                            phi_kT[hp][hb:hb + 64, cs], ident[hb:hb + 64, hb:hb + 64],
                            start=False, stop=(i == 1),
                        )
                    a_sb = attn.tile([128, 2, 128], bf16, tag=f"Asb{par}",
                                     name=f"a_sb_{c}_{par}")
                    nc.vector.tensor_tensor(
                        a_sb[:], a_ps[:, 0:256].rearrange("p (g t) -> p g t", g=2),
                        maskT2[:], op=Alu.mult)
                    # phi_k seq-major: psum fp32 -> sbuf bf16, parity plane
                    # (Act engine both times: DVE is the binding engine in the
                    # attention-dense stretch)
                    nc.scalar.activation(
                        phi_ks[:, c, :, par, :],
                        a_ps[:, 256:384].rearrange("p (i e) -> p i e", i=2),
                        Act.Copy)
                    st["aop"][(c, par)] = a_sb

            def attn_back(c):
                cs = slice(c * 128, (c + 1) * 128)
                S_sb = st["S_sb"]
                op_ps = {}
                for par in range(2):
                    hb = 64 * par
                    a_sb = st["aop"].pop((c, par))
                    op = ps_op.tile([128, 130], f32, tag=f"op{par}",
                                    name=f"op_ps_{c}_{par}")
                    for i, h in enumerate((par, par + 2)):
                        hp = h // 2
                        nc.tensor.matmul(
                            op[:, i * 65:(i + 1) * 65],
                            a_sb[:, i, :], v_aug[:, c, h, :],
                            start=(i == 0), stop=(c == 0 and i == 1),
                        )
                        if c > 0:
                            nc.tensor.matmul(
                                op[:, i * 65:(i + 1) * 65],
                                phi_qT[hp][hb:hb + 64, cs],
                                S_sb[hb:hb + 64, 130 * hp + 65 * par:
                                     130 * hp + 65 * par + 65],
                                start=False, stop=(i == 1),
                            )
                    op_ps[par] = op
                # state increment for chunk c (PE, one long group, base 0)
                for hp in range(2):
                    nc.tensor.matmul(
                        S_ps[:, 130 * hp:130 * (hp + 1)],
                        phi_ks[:, c, hp].rearrange("p a e -> p (a e)"),
                        v_aug[:, c, 2 * hp:2 * hp + 2, :],
                        start=(c == 0 and hp == 0),
                        stop=(c == NCHUNK - 1 and hp == 1),
                    )
                # snapshot state for chunk c+1 (Act): psum fp32 -> sbuf bf16
                if c < NCHUNK - 1:
                    S_new = attn.tile([128, 2 * 130], bf16, tag="Ssb",
                                      name=f"S_sb_{c}")
                    nc.scalar.activation(S_new[:], S_ps[:], Act.Copy)
                    st["S_sb"] = S_new
                # normalize (DVE): rcp of the ones-column, then scale
                qt, cc = c // CPQ, c % CPQ
                if cc == 0:
                    st["o_sb"] = work.tile([128, CPQ, 2, 2, 64], bf16, tag="osb",
                                           name=f"o_sb_{qt}", bufs=2)
                o_sb = st["o_sb"]
                for par in range(2):
                    op4 = op_ps[par].rearrange("p (i e) -> p i e", i=2)
                    rcp = attn.tile([128, 2], f32, tag=f"rcp{par}",
                                    name=f"rcp_{c}_{par}")
                    nc.vector.reciprocal(rcp[:], op4[:, :, 64])
                    nc.vector.tensor_tensor(
                        o_sb[:, cc, :, par, :], op4[:, :, 0:64],
                        rcp[:].broadcast_to([128, 2, 64]),
                        op=Alu.mult)
                out_q = out.rearrange("(q c p) he -> q p c he", c=CPQ, p=128)[qt]
                if qt == NQ - 1:
                    # tail: per-chunk stores, alternating trigger queues so the
                    # HWDGE/DGE latencies of consecutive stores overlap
                    eng = nc.sync if cc % 2 == 0 else nc.scalar
                    eng.dma_start(
                        out_q[:, cc:cc + 1, :],
                        o_sb.rearrange("p c a b e -> p c (a b e)")[:, cc:cc + 1, :])
                elif cc == CPQ - 1:
                    nc.sync.dma_start(
                        out_q, o_sb.rearrange("p c a b e -> p c (a b e)")[:])

            # ---- global schedule ----
            # qt0: projections only (DMA-starved anyway). qt1/qt2: previous
            # quarter's 4 chunks at half-unit pace. qt3: chunks 8-15 (quarter
            # 2's and its own) at one per unit with a one-slot F/B skew, so
            # only chunk 15's back half trails the final projection.
            def units_of(qt):
                return [qk_unit(qt, "q", phi_qT, 0), qk_unit(qt, "q", phi_qT, 1),
                        qk_unit(qt, "k", phi_kT, 0), qk_unit(qt, "k", phi_kT, 1),
                        v_unit(qt, 0), v_unit(qt, 1), v_unit(qt, 2), v_unit(qt, 3)]

            import os
            W0, W1, W2, W3 = (int(v) for v in os.environ.get(
                "KWARM", "8,6,8,10").split(","))
            F_OFF, B_OFF = (int(v) for v in os.environ.get(
                "KSCHED", "6,8").split(","))
            # slot plan: unit u emitted at slot u; F(c) at slot
            # 8*(c//4)+F_OFF+(c%4), B(c) at 8*(c//4)+B_OFF+(c%4).
            # Backs emit before fronts within a slot (keeps the S-snapshot
            # copy at the head of the Act queue).
            slots = {}
            for c in range(NCHUNK):
                qc, cc = c // CPQ, c % CPQ
                slots.setdefault(min(8 * qc + F_OFF + cc, 31), []).append(("F", c))
                slots.setdefault(min(8 * qc + B_OFF + cc, 33), []).append(("B", c))
            units = [u for qt in range(NQ) for u in units_of(qt)]
            warm(W0)  # pre-first-matmul warmup
            for slot in range(34):
                if slot < 32:
                    units[slot]()
                if slot == 3:
                    warm(W1)  # k1(0) -> v0(0) xv0a wait
                elif slot == 5:
                    warm(W2)  # v1(0) -> v2(0) xv0b wait
                elif slot == 7:
                    warm(W3)  # qt0 -> qt1 xq1 wait
                    warm_budget[0] = False
                for kind, c in sorted(slots.get(slot, [])):
                    (attn_back if kind == "B" else attn_front)(c)

    nc.compile()
    return nc


def _get_nc():
    with _lock:
        if "nc" not in _cache:
            _cache["nc"] = _build_nc()
        return _cache["nc"]


def kernel(query, key, value, query_kernel, key_kernel, value_kernel):
    import ml_dtypes
    from concourse.bass_utils import run_bass_kernel_spmd

    nc = _get_nc()
    bf16 = ml_dtypes.bfloat16

    xT = {}
    for b in range(B):
        xT[("q", b)] = np.ascontiguousarray(query[b].T.astype(bf16))
        xT[("k", b)] = np.ascontiguousarray(key[b].T.astype(bf16))
        xT[("v", b)] = np.ascontiguousarray(value[b].T.astype(bf16))

    def w_arrange(wk_full, h0):
        w = wk_full[:, h0:h0 + HPC, :].reshape(D, HDC)  # [D, HDC]
        # [p, dc, m] with p = d % 128, dc = d // 128
        return np.ascontiguousarray(
            w.reshape(DC, 128, HDC).transpose(1, 0, 2).astype(bf16))

    in_maps = []
    for c in range(N_CORES):
        b, h0 = c // 4, 4 * (c % 4)
        in_maps.append({
            "xqT": xT[("q", b)],
            "xkT": xT[("k", b)],
            "xvT": xT[("v", b)],
            "wq": w_arrange(query_kernel, h0),
            "wk": w_arrange(key_kernel, h0),
            "wv": w_arrange(value_kernel, h0),
        })

    results = run_bass_kernel_spmd(nc, in_maps, core_ids=list(range(N_CORES)))

    # The reference ends with a FLAT reshape of [B*H, S, HD] -> (B, S, H*HD):
    # output rows [128h:128h+128] of batch b are head h's [S, HD] attention
    # output flat-reshaped to [128, H*HD].
    full = np.empty((B, S, H * HD), dtype=np.float32)
    for c in range(N_CORES):
        b, h0 = c // 4, 4 * (c % 4)
        av = np.asarray(results.results[c]["out"]).astype(np.float32).reshape(S, HPC, HD)
        for hl in range(HPC):
            full[b, (h0 + hl) * 128:(h0 + hl + 1) * 128, :] = (
                av[:, hl, :].reshape(128, H * HD))
    return full
